# revision 8
# baseline (speedup 1.0000x reference)
"""Causal self-attention (B=1, L=4096, C=1024, H=16, D=64) on 8 TRN2 NeuronCores.

Sharding: head tensor-parallelism — each core owns 2 of the 16 heads.
Host passes per-core bf16 inputs: x in block-major layout [8, 128, 4096],
the Wq/Wk/Wv column slices (chunk-major) and the Wo row slice for the
core's heads. Each core computes its partial of
out.T = Wo_local.T @ attn_local.T; the host sums the 8 partials.

The kernel is PE-bound (~175us of matmul streaming at the 2.4GHz full
p-state), so everything is organized to keep the tensor engine
continuously fed and everyone else off its critical path:
  - x is DMA'd per 512-column L-block on two hwdge queues (SP + ACT) in
    host-prepacked contiguous 8KB-per-partition lines; block 0 is split
    per 128-chunk so the first projection matmul starts as early as
    possible after the multi-core start barrier.
  - S^T for both heads of one k-block goes into a single 2-bank PSUM tile
    [128, 2, 512]; ONE ACT exp instruction covers both heads (144 instead
    of 288 activates), keeping the scalar engine comfortably ahead.
  - No pre-exp additive mask: exp runs unmasked (|scores/8| < ~8, no
    overflow) and the upper triangle of P^T is zeroed post-exp on the
    otherwise-idle Pool engine via affine_select.
  - O matmuls trail S/exp by 2 iterations (software pipeline).
  - A budget-based filler queue drops self-contained PE work units
    (projection chains for block b+2, V transposes, and the previous
    blocks' Wo matmuls) between attention iterations, absorbing the
    ~200ns/iteration PE slack left by the exp pacing and keeping the
    PE p-state pinned at full speed.
  - Epilogue normalize is staggered: o_ps drain + reciprocal (DVE) right
    at the next block's start, the broadcast matmuls 6 iterations later
    (after the ~5us DVE reciprocal chain has drained), and the Wo
    matmuls spread as fillers.
  - softmax denominators: row 64 of the O accumulation (vaug ones row);
    both heads' denominators sit at partitions 0/64 of one [65, 512]
    tile so a single DVE reciprocal (cost is per-free-element) serves
    both; broadcast via a bf16 rank-1 matmul.
"""
import math
import sys
from collections import deque
from contextlib import ExitStack

import numpy as np

sys.path.insert(0, "/opt/trn_rl_repo")

import ml_dtypes  # noqa: E402

import concourse.bass as bass  # noqa: E402,F401
import concourse.mybir as mybir  # noqa: E402
import concourse.tile as tile  # noqa: E402
from concourse import bacc  # noqa: E402

FP32 = mybir.dt.float32
BF16 = mybir.dt.bfloat16

L, C, H, D = 4096, 1024, 16, 64
N_CORES = 8
NQ_G = L // 512


def _build_nc():
    DH2, QB, KB = 128, 512, 128
    NQ = L // QB          # 8 q-blocks
    NCC = C // 128        # 8 contraction chunks
    SUB = QB // KB        # 4 k-blocks per q-block width
    NKB = L // KB         # 32 k-blocks
    scale = 1.0 / math.sqrt(D)
    Exp = mybir.ActivationFunctionType.Exp

    nc = bacc.Bacc("TRN2", target_bir_lowering=False, debug=False,
                   num_devices=N_CORES)
    xb = nc.declare_dram_parameter("xb", [NQ_G, 128, C // 128 * 512], BF16,
                                   isOutput=False)
    # [128, NCC*DH2] chunk-major (host pre-reshaped) -> one contiguous DMA
    wq = nc.declare_dram_parameter("wq", [128, C], BF16, isOutput=False)
    wk = nc.declare_dram_parameter("wk", [128, C], BF16, isOutput=False)
    wv = nc.declare_dram_parameter("wv", [128, C], BF16, isOutput=False)
    wo = nc.declare_dram_parameter("wo", [DH2, C], BF16, isOutput=False)
    outT = nc.declare_dram_parameter("outT", [C, L], BF16, isOutput=True)

    with tile.TileContext(nc) as tc, ExitStack() as ctx:
        big = ctx.enter_context(tc.tile_pool(name="big", bufs=1))
        work = ctx.enter_context(tc.tile_pool(name="work", bufs=4))
        ptp = ctx.enter_context(tc.tile_pool(name="ptp", bufs=6))
        # PSUM: st 2 bufs x 2 banks + o0/o1 1 buf x 1 bank + misc 2 x 1 = 8
        ps_st = ctx.enter_context(tc.tile_pool(name="ps_st", bufs=2,
                                               space="PSUM"))
        ps_o = ctx.enter_context(tc.tile_pool(name="ps_o", bufs=1,
                                              space="PSUM"))
        ps_m = ctx.enter_context(tc.tile_pool(name="ps_m", bufs=2,
                                              space="PSUM"))

        ident = big.tile([128, 128], BF16, tag="ident")
        nc.gpsimd.memset(ident[:], 0.0)
        nc.gpsimd.affine_select(out=ident[:], in_=ident[:],
                                compare_op=mybir.AluOpType.not_equal,
                                fill=1.0, base=0,
                                pattern=[[-1, 128]], channel_multiplier=1)
        ones64 = big.tile([1, D], BF16, tag="ones64")
        nc.gpsimd.memset(ones64[:], 1.0)

        xt_sb = big.tile([128, NQ, NCC, QB], BF16, tag="xt")
        wq_sb = big.tile([128, NCC, DH2], BF16, tag="wq")
        wk_sb = big.tile([128, NCC, DH2], BF16, tag="wk")
        wv_sb = big.tile([128, NCC, DH2], BF16, tag="wv")
        wo_sb = big.tile([128, C], BF16, tag="wo")
        # weights + odd x-blocks on the ACT hwdge queue; even x-blocks and
        # output stores on the SP queue — two DMA queues run concurrently.
        for w_sb, w_dram in ((wq_sb, wq), (wk_sb, wk), (wv_sb, wv)):
            nc.scalar.dma_start(
                w_sb[:], w_dram.rearrange("p (n d) -> p n d", n=NCC))
        nc.scalar.dma_start(wo_sb[:], wo[:])
        for c in range(NCC):  # block 0 per-chunk: earliest possible start
            nc.sync.dma_start(xt_sb[:, 0, c, :],
                              xb[0][:, c * QB:(c + 1) * QB])
        for b in range(1, NQ):
            eng = nc.sync if b % 2 == 0 else nc.scalar
            eng.dma_start(xt_sb[:, b],
                          xb[b].rearrange("p (n q) -> p n q", n=NCC))

        qt2 = big.tile([128, L], BF16, tag="qt2")
        kt2 = big.tile([128, L], BF16, tag="kt2")
        vt2 = big.tile([128, L], BF16, tag="vt2")
        vaug = big.tile([128, NKB, 130], BF16, tag="vaug")
        nc.gpsimd.memset(vaug[:, :, 64:65], 1.0)
        nc.gpsimd.memset(vaug[:, :, 129:130], 1.0)

        def emit_proj(dst, w_sb, b):
            pp = ps_m.tile([128, QB], FP32, tag="m", name="pp")
            for c in range(NCC):
                nc.tensor.matmul(pp[:], w_sb[:, c, :], xt_sb[:, b, c, :],
                                 start=(c == 0), stop=(c == NCC - 1))
            nc.vector.tensor_copy(dst[:, b * QB:(b + 1) * QB], pp[:])

        def emit_vtrans1(i):
            trp = ps_m.tile([128, 128], BF16, tag="m", name="trp")
            nc.tensor.transpose(trp[:], vt2[:, i * KB:(i + 1) * KB],
                                ident[:])
            nc.vector.tensor_copy(vaug[:, i, 0:64], trp[:, 0:64])
            nc.vector.tensor_copy(vaug[:, i, 65:129], trp[:, 64:128])

        # ---- filler scheduler ------------------------------------------
        # entries: (cost_ns, proj_target_block_or_None, emit_fn); each is
        # an atomic PE work unit (keeps the ps_m 2-buffer rotation safe).
        fillers = deque()
        budget = [0]
        SLACK_NS = 220

        def push_proj_fillers(b):
            fillers.append((1900, b, lambda: emit_proj(kt2, wk_sb, b)))
            fillers.append((1900, b, lambda: emit_proj(qt2, wq_sb, b)))
            fillers.append((1900, b, lambda: emit_proj(vt2, wv_sb, b)))
            for s in range(SUB):
                fillers.append((350, b, lambda i=b * SUB + s: emit_vtrans1(i)))

        def drop_fillers():
            while fillers and budget[0] >= fillers[0][0]:
                cost, _, fn = fillers.popleft()
                budget[0] -= cost
                fn()

        def flush_fillers(upto_block=None):
            def pending_proj():
                return any(t is not None and t <= upto_block
                           for _, t, _ in fillers)
            while fillers and (upto_block is None or pending_proj()):
                fillers.popleft()[2]()
            budget[0] = 0

        for b in (0, 1):
            push_proj_fillers(b)
        flush_fillers()

        o_ps_cur = [None, None]
        epi = {}

        def emit_epi_drain(j):
            # o_ps drain: DVE-only, frees the o banks fast. The reciprocal
            # is deferred (emit_epi_recip) so filler copies dropped in the
            # next few iterations aren't queued behind its 3.3us.
            den2 = work.tile([65, QB], FP32, tag="den2")
            osbs = []
            for h in range(2):
                osb = work.tile([64, QB], FP32, tag="osb", name=f"osb{h}")
                nc.vector.tensor_copy(osb[:], o_ps_cur[h][0:64, :])
                nc.vector.tensor_copy(den2[h * 64:h * 64 + 1, :],
                                      o_ps_cur[h][64:65, :])  # frees bank
                osbs.append(osb)
            epi.update(osbs=osbs, den2=den2)

        def emit_epi_recip():
            rc2 = work.tile([65, QB], FP32, tag="rc2")
            nc.vector.reciprocal(rc2[:], epi["den2"][:])
            rcbs = []
            for h in range(2):
                rcb = work.tile([1, QB], BF16, tag="rcb", name=f"rcb{h}")
                nc.vector.tensor_copy(rcb[:], rc2[h * 64:h * 64 + 1, :])
                rcbs.append(rcb)
            epi["rcbs"] = rcbs

        def emit_epi_bc():
            att2 = work.tile([128, QB], BF16, tag="att2")
            for h in range(2):
                bc = ps_m.tile([64, QB], FP32, tag="m", name="bc")
                nc.tensor.matmul(bc[:], ones64[:], epi["rcbs"][h][:],
                                 start=True, stop=True)
                nc.vector.tensor_mul(att2[h * D:(h + 1) * D, :],
                                     epi["osbs"][h][:], bc[:])
            epi["att2"] = att2

        def push_wo_fillers(j):
            att2 = epi["att2"]
            for cc in range(NCC):
                def unit(cc=cc, att2=att2, j=j):
                    op = ps_m.tile([128, QB], FP32, tag="m", name="op")
                    nc.tensor.matmul(op[:],
                                     wo_sb[:, cc * 128:(cc + 1) * 128],
                                     att2[:], start=True, stop=True)
                    ot = work.tile([128, QB], BF16, tag="ot")
                    if cc % 2 == 0:
                        nc.scalar.copy(ot[:], op[:])
                    else:
                        nc.vector.tensor_copy(ot[:], op[:])
                    nc.sync.dma_start(
                        outT[cc * 128:(cc + 1) * 128,
                             j * QB:(j + 1) * QB], ot[:])
                fillers.append((450, None, unit))

        def emit_final_epilogue(j):
            # nothing follows the last block: pipeline the normalize chain
            # at half width so the PE pieces overlap the DVE reciprocal.
            HB = QB // 2
            den2 = work.tile([65, QB], FP32, tag="den2")
            for h in range(2):
                nc.vector.tensor_copy(den2[h * 64:h * 64 + 1, :],
                                      o_ps_cur[h][64:65, :])
            rc2 = work.tile([65, QB], FP32, tag="rc2")
            rcbs = [[None, None], [None, None]]
            nc.vector.reciprocal(rc2[:, 0:HB], den2[:, 0:HB])
            for h in range(2):
                rcb = work.tile([1, HB], BF16, tag="rcbf", name=f"rcbf0{h}")
                nc.vector.tensor_copy(rcb[:], rc2[h * 64:h * 64 + 1, 0:HB])
                rcbs[0][h] = rcb
            osbs = []
            for h in range(2):
                osb = work.tile([64, QB], FP32, tag="osb", name=f"osbf{h}")
                nc.vector.tensor_copy(osb[:], o_ps_cur[h][0:64, :])
                osbs.append(osb)
            nc.vector.reciprocal(rc2[:, HB:QB], den2[:, HB:QB])
            for h in range(2):
                rcb = work.tile([1, HB], BF16, tag="rcbf", name=f"rcbf1{h}")
                nc.vector.tensor_copy(rcb[:], rc2[h * 64:h * 64 + 1, HB:QB])
                rcbs[1][h] = rcb
            att2 = work.tile([128, QB], BF16, tag="att2")
            for half in range(2):
                cs = slice(half * HB, (half + 1) * HB)
                for h in range(2):
                    bc = ps_m.tile([64, HB], FP32, tag="m", name="bcf")
                    nc.tensor.matmul(bc[:], ones64[:], rcbs[half][h][:],
                                     start=True, stop=True)
                    nc.vector.tensor_mul(att2[h * D:(h + 1) * D, cs],
                                         osbs[h][:, cs], bc[:])
                for cc in range(NCC):
                    op = ps_m.tile([128, HB], FP32, tag="m", name="opf")
                    nc.tensor.matmul(op[:],
                                     wo_sb[:, cc * 128:(cc + 1) * 128],
                                     att2[:, cs], start=True, stop=True)
                    ot = work.tile([128, HB], BF16, tag="otf", name="otf")
                    if cc % 2 == 0:
                        nc.scalar.copy(ot[:], op[:])
                    else:
                        nc.vector.tensor_copy(ot[:], op[:])
                    nc.sync.dma_start(
                        outT[cc * 128:(cc + 1) * 128,
                             j * QB + half * HB:j * QB + (half + 1) * HB],
                        ot[:])

        prev = [None]
        for j in range(NQ):
            if j + 2 < NQ:
                push_proj_fillers(j + 2)
            nk = (j + 1) * SUB
            pend = []
            o_ps_new = None

            def emit_O(item, last_i):
                oi, oc0, opt = item
                for h in range(2):
                    nc.tensor.matmul(o_ps_new[h][:, oc0:QB],
                                     vaug[:, oi, h * 65:h * 65 + 65],
                                     opt[:, h, oc0:QB],
                                     start=(oi == 0), stop=(oi == last_i))

            for i in range(nk):
                c0 = max(0, i - j * SUB) * KB
                st = ps_st.tile([128, 2, QB], FP32, tag="st")
                for h in range(2):
                    r0, r1 = h * D, (h + 1) * D
                    nc.tensor.matmul(st[:, h, c0:QB],
                                     kt2[r0:r1, i * KB:(i + 1) * KB],
                                     qt2[r0:r1, j * QB + c0:(j + 1) * QB],
                                     start=True, stop=True)
                pt = ptp.tile([128, 2, QB], BF16, tag="pt")
                nc.scalar.activation(pt[:, :, c0:QB], st[:, :, c0:QB], Exp,
                                     scale=scale)
                if i >= j * SUB:
                    for h in range(2):
                        nc.gpsimd.affine_select(
                            out=pt[:, h, c0:c0 + KB],
                            in_=pt[:, h, c0:c0 + KB],
                            compare_op=mybir.AluOpType.is_ge, fill=0.0,
                            base=0, pattern=[[1, KB]], channel_multiplier=-1)
                pend.append((i, c0, pt))
                # previous block's epilogue, staggered: the o_ps drain must
                # precede this block's first O acquisition at i==2 (ps_o has
                # bufs=1 -> WAR order), and the bc matmuls must trail the
                # ~5us DVE reciprocal chain or they head-of-line-block the
                # PE stream.
                if i == 0 and prev[0] is not None:
                    emit_epi_drain(prev[0])
                elif i == 3 and prev[0] is not None:
                    emit_epi_recip()
                elif i == 6 and prev[0] is not None:
                    emit_epi_bc()
                    push_wo_fillers(prev[0])
                    prev[0] = None
                if len(pend) > 2:
                    if o_ps_new is None:
                        o_ps_new = [ps_o.tile([65, QB], FP32, tag=f"o{h}",
                                              name=f"o_ps{h}")
                                    for h in range(2)]
                    emit_O(pend.pop(0), nk - 1)
                budget[0] += SLACK_NS
                drop_fillers()
            for item in pend:
                emit_O(item, nk - 1)
            # projections for block j+1 must be complete before it starts
            flush_fillers(upto_block=j + 1)
            o_ps_cur = o_ps_new
            prev[0] = j
        if prev[0] is not None:  # j=7's bc never hit i==6? (it always does)
            pass
        flush_fillers()
        emit_final_epilogue(NQ - 1)
    nc.compile()
    return nc


_NC_CACHE = None


def _get_nc():
    global _NC_CACHE
    if _NC_CACHE is None:
        _NC_CACHE = _build_nc()
    return _NC_CACHE


def _chunk_major(w):
    """[1024, 128] -> [128, 8*128]: element [p, n*128+d] = w[n*128+p, d]."""
    return np.ascontiguousarray(
        w.reshape(8, 128, 128).transpose(1, 0, 2).reshape(128, 1024))


def make_in_maps(x, Wq, Wk, Wv, Wo):
    bf16 = ml_dtypes.bfloat16
    x = np.asarray(x, np.float32).reshape(L, C)
    # xb[b, p, c*512 + l] = x[b*512 + l, c*128 + p]: per 512-column L-block,
    # per-partition-contiguous 8KB lines -> one full-speed DMA per block.
    xb = np.ascontiguousarray(
        x.reshape(8, 512, 8, 128).transpose(0, 3, 2, 1).reshape(8, 128, 4096)
    ).astype(bf16)
    Wq, Wk, Wv, Wo = (np.asarray(w, np.float32) for w in (Wq, Wk, Wv, Wo))
    in_maps = []
    for c in range(N_CORES):
        cols = slice(128 * c, 128 * (c + 1))
        in_maps.append({
            "xb": xb,
            "wq": _chunk_major(Wq[:, cols]).astype(bf16),
            "wk": _chunk_major(Wk[:, cols]).astype(bf16),
            "wv": _chunk_major(Wv[:, cols]).astype(bf16),
            "wo": np.ascontiguousarray(Wo[cols, :]).astype(bf16),
        })
    return in_maps


def combine_results(results):
    acc = np.zeros((C, L), np.float32)
    for r in results:
        acc += np.asarray(r["outT"], np.float32)
    return np.ascontiguousarray(acc.T)[None].astype(np.float32)


def kernel(x, Wq, Wk, Wv, Wo):
    from concourse.bass_utils import run_bass_kernel_spmd
    nc = _get_nc()
    in_maps = make_in_maps(x, Wq, Wk, Wv, Wo)
    res = run_bass_kernel_spmd(nc, in_maps, core_ids=list(range(N_CORES)))
    return combine_results(res.results)


# revision 9
# speedup vs baseline: 1.0285x; 1.0285x over previous
"""Causal self-attention (B=1, L=4096, C=1024, H=16, D=64) on 8 TRN2 NeuronCores.

Sharding: head tensor-parallelism — each core owns 2 of the 16 heads.
Host passes per-core bf16 inputs: x transposed [C, L], the Wq/Wk/Wv column
slices and Wo row slice for the core's heads. Each core computes its partial
of out.T = Wo_local.T @ attn_local.T; the host sums the 8 partials.

Schedule (v2): the kernel is PE-bound (~175us of matmul streaming at the
2.4GHz full p-state), so everything is organized to keep the tensor engine
continuously fed and everyone else off its critical path:
  - x is DMA'd per 512-column L-block on two hwdge queues (SP + ACT), so
    the first projection starts ~4.5us in instead of after the full 8MB.
  - QKV projection + V-transpose work for block b+2 is emitted as filler
    tasks interleaved into attention block b, hiding projections inside
    the attention middle.
  - S^T for both heads of one k-block goes into a single 2-bank PSUM tile
    [128, 2, 512]; ONE ACT exp instruction covers both heads (144 instead
    of 288 activates), keeping the scalar engine comfortably ahead.
  - No pre-exp additive mask: exp runs unmasked (|scores/8| < ~8, no
    overflow) and the upper triangle of P^T is zeroed post-exp on the
    otherwise-idle Pool engine via affine_select.
  - O matmuls trail S/exp by 2 iterations (software pipeline) and each
    q-block's epilogue (normalize + Wo + store) is emitted inside the
    next block's first iterations, so the PE always has ready work.
  - softmax denominators: row 64 of the O accumulation (vaug ones row),
    reciprocal via the fast DVE approximation, broadcast via a bf16
    rank-1 matmul.
"""
import math
import sys
from collections import deque
from contextlib import ExitStack

import numpy as np

sys.path.insert(0, "/opt/trn_rl_repo")

import ml_dtypes  # noqa: E402

import concourse.bass as bass  # noqa: E402,F401
import concourse.mybir as mybir  # noqa: E402
import concourse.tile as tile  # noqa: E402
from concourse import bacc  # noqa: E402

FP32 = mybir.dt.float32
BF16 = mybir.dt.bfloat16

L, C, H, D = 4096, 1024, 16, 64
N_CORES = 8


def _build_nc():
    DH2, QB, KB = 128, 512, 128
    NQ = L // QB          # 8 q-blocks
    NCC = C // 128        # 8 contraction chunks
    SUB = QB // KB        # 4 k-blocks per q-block width
    NKB = L // KB         # 32 k-blocks
    scale = 1.0 / math.sqrt(D)
    Exp = mybir.ActivationFunctionType.Exp

    nc = bacc.Bacc("TRN2", target_bir_lowering=False, debug=False,
                   num_devices=N_CORES)
    xT = nc.declare_dram_parameter("xT", [C, L], BF16, isOutput=False)
    # [128, NCC*DH2] chunk-major (host pre-reshaped) -> one contiguous DMA
    wq = nc.declare_dram_parameter("wq", [128, C], BF16, isOutput=False)
    wk = nc.declare_dram_parameter("wk", [128, C], BF16, isOutput=False)
    wv = nc.declare_dram_parameter("wv", [128, C], BF16, isOutput=False)
    wo = nc.declare_dram_parameter("wo", [DH2, C], BF16, isOutput=False)
    outT = nc.declare_dram_parameter("outT", [C, L], BF16, isOutput=True)

    with tile.TileContext(nc) as tc, ExitStack() as ctx:
        big = ctx.enter_context(tc.tile_pool(name="big", bufs=1))
        work = ctx.enter_context(tc.tile_pool(name="work", bufs=4))
        ptp = ctx.enter_context(tc.tile_pool(name="ptp", bufs=6))
        # PSUM: st 2 bufs x 2 banks + o0/o1 1 buf x 1 bank + misc 2 x 1 = 8
        ps_st = ctx.enter_context(tc.tile_pool(name="ps_st", bufs=2,
                                               space="PSUM"))
        ps_o = ctx.enter_context(tc.tile_pool(name="ps_o", bufs=1,
                                              space="PSUM"))
        ps_m = ctx.enter_context(tc.tile_pool(name="ps_m", bufs=2,
                                              space="PSUM"))

        ident = big.tile([128, 128], BF16, tag="ident")
        nc.gpsimd.memset(ident[:], 0.0)
        nc.gpsimd.affine_select(out=ident[:], in_=ident[:],
                                compare_op=mybir.AluOpType.not_equal,
                                fill=1.0, base=0,
                                pattern=[[-1, 128]], channel_multiplier=1)
        ones64 = big.tile([1, D], BF16, tag="ones64")
        nc.gpsimd.memset(ones64[:], 1.0)

        xt_sb = big.tile([128, NCC, L], BF16, tag="xt")
        wq_sb = big.tile([128, NCC, DH2], BF16, tag="wq")
        wk_sb = big.tile([128, NCC, DH2], BF16, tag="wk")
        wv_sb = big.tile([128, NCC, DH2], BF16, tag="wv")
        wo_sb = big.tile([128, C], BF16, tag="wo")
        # weights + odd x-blocks on the ACT hwdge queue; even x-blocks and
        # output stores on the SP queue — two DMA queues run concurrently.
        for w_sb, w_dram in ((wq_sb, wq), (wk_sb, wk), (wv_sb, wv)):
            nc.scalar.dma_start(
                w_sb[:], w_dram.rearrange("p (n d) -> p n d", n=NCC))
        nc.scalar.dma_start(wo_sb[:], wo[:])
        xTr = xT.rearrange("(n p) l -> p n l", p=128)
        for b in range(NQ):
            eng = nc.sync if b % 2 == 0 else nc.scalar
            eng.dma_start(xt_sb[:, :, b * QB:(b + 1) * QB],
                          xTr[:, :, b * QB:(b + 1) * QB])

        qt2 = big.tile([128, L], BF16, tag="qt2")
        kt2 = big.tile([128, L], BF16, tag="kt2")
        vt2 = big.tile([128, L], BF16, tag="vt2")
        vaug = big.tile([128, NKB, 130], BF16, tag="vaug")
        nc.gpsimd.memset(vaug[:, :, 64:65], 1.0)
        nc.gpsimd.memset(vaug[:, :, 129:130], 1.0)

        def emit_proj(dst, w_sb, b):
            pp = ps_m.tile([128, QB], FP32, tag="m", name="pp")
            for c in range(NCC):
                nc.tensor.matmul(pp[:], w_sb[:, c, :],
                                 xt_sb[:, c, b * QB:(b + 1) * QB],
                                 start=(c == 0), stop=(c == NCC - 1))
            nc.vector.tensor_copy(dst[:, b * QB:(b + 1) * QB], pp[:])

        def emit_vtrans(b):
            for s in range(SUB):
                i = b * SUB + s
                trp = ps_m.tile([128, 128], BF16, tag="m", name="trp")
                nc.tensor.transpose(trp[:], vt2[:, i * KB:(i + 1) * KB],
                                    ident[:])
                nc.vector.tensor_copy(vaug[:, i, 0:64], trp[:, 0:64])
                nc.vector.tensor_copy(vaug[:, i, 65:129], trp[:, 64:128])

        tasks = deque()

        def push_tasks(b):
            tasks.append(lambda: emit_proj(kt2, wk_sb, b))
            tasks.append(lambda: emit_proj(qt2, wq_sb, b))
            tasks.append(lambda: emit_proj(vt2, wv_sb, b))
            tasks.append(lambda: emit_vtrans(b))

        for b in (0, 1):
            push_tasks(b)
        while tasks:
            tasks.popleft()()

        o_ps_cur = [None, None]

        def emit_epilogue(j):
            att2 = work.tile([128, QB], BF16, tag="att2")
            den2 = work.tile([65, QB], FP32, tag="den2")
            osbs = []
            for h in range(2):
                osb = work.tile([64, QB], FP32, tag="osb", name=f"osb{h}")
                nc.vector.tensor_copy(osb[:], o_ps_cur[h][0:64, :])
                nc.vector.tensor_copy(den2[h * 64:h * 64 + 1, :],
                                      o_ps_cur[h][64:65, :])  # frees bank
                osbs.append(osb)
            rc2 = work.tile([65, QB], FP32, tag="rc2")
            nc.vector.reciprocal(rc2[:], den2[:])
            for h in range(2):
                rcb = work.tile([1, QB], BF16, tag="rcb", name=f"rcb{h}")
                nc.vector.tensor_copy(rcb[:], rc2[h * 64:h * 64 + 1, :])
                bc = ps_m.tile([64, QB], FP32, tag="m", name="bc")
                nc.tensor.matmul(bc[:], ones64[:], rcb[:],
                                 start=True, stop=True)
                nc.vector.tensor_mul(att2[h * D:(h + 1) * D, :],
                                     osbs[h][:], bc[:])
            for cc in range(NCC):
                op = ps_m.tile([128, QB], FP32, tag="m", name="op")
                nc.tensor.matmul(op[:], wo_sb[:, cc * 128:(cc + 1) * 128],
                                 att2[:], start=True, stop=True)
                ot = work.tile([128, QB], BF16, tag="ot")
                nc.vector.tensor_copy(ot[:], op[:])
                nc.sync.dma_start(
                    outT[cc * 128:(cc + 1) * 128, j * QB:(j + 1) * QB], ot[:])

        prev_j = None
        for j in range(NQ):
            if j + 2 < NQ:
                push_tasks(j + 2)
            nk = (j + 1) * SUB
            step = max(1, nk // 4)
            pend = []
            o_ps_new = None

            def emit_O(item, last_i):
                oi, oc0, opt = item
                for h in range(2):
                    nc.tensor.matmul(o_ps_new[h][:, oc0:QB],
                                     vaug[:, oi, h * 65:h * 65 + 65],
                                     opt[:, h, oc0:QB],
                                     start=(oi == 0), stop=(oi == last_i))

            for i in range(nk):
                c0 = max(0, i - j * SUB) * KB
                st = ps_st.tile([128, 2, QB], FP32, tag="st")
                for h in range(2):
                    r0, r1 = h * D, (h + 1) * D
                    nc.tensor.matmul(st[:, h, c0:QB],
                                     kt2[r0:r1, i * KB:(i + 1) * KB],
                                     qt2[r0:r1, j * QB + c0:(j + 1) * QB],
                                     start=True, stop=True)
                pt = ptp.tile([128, 2, QB], BF16, tag="pt")
                nc.scalar.activation(pt[:, :, c0:QB], st[:, :, c0:QB], Exp,
                                     scale=scale)
                if i >= j * SUB:
                    for h in range(2):
                        nc.gpsimd.affine_select(
                            out=pt[:, h, c0:c0 + KB],
                            in_=pt[:, h, c0:c0 + KB],
                            compare_op=mybir.AluOpType.is_ge, fill=0.0,
                            base=0, pattern=[[1, KB]], channel_multiplier=-1)
                pend.append((i, c0, pt))
                # previous block's epilogue before this block's first O
                # acquisition: ps_o has bufs=1, so the WAR order matters.
                if i == 1 and prev_j is not None:
                    emit_epilogue(prev_j)
                if len(pend) > 2:
                    if o_ps_new is None:
                        o_ps_new = [ps_o.tile([65, QB], FP32, tag=f"o{h}",
                                              name=f"o_ps{h}")
                                    for h in range(2)]
                    emit_O(pend.pop(0), nk - 1)
                if tasks and i % step == step - 1:
                    tasks.popleft()()
            for item in pend:
                emit_O(item, nk - 1)
            o_ps_cur = o_ps_new
            prev_j = j
        emit_epilogue(NQ - 1)
    nc.compile()
    return nc


_NC_CACHE = None


def _get_nc():
    global _NC_CACHE
    if _NC_CACHE is None:
        _NC_CACHE = _build_nc()
    return _NC_CACHE


def _chunk_major(w):
    """[1024, 128] -> [128, 8*128]: element [p, n*128+d] = w[n*128+p, d]."""
    return np.ascontiguousarray(
        w.reshape(8, 128, 128).transpose(1, 0, 2).reshape(128, 1024))


def make_in_maps(x, Wq, Wk, Wv, Wo):
    bf16 = ml_dtypes.bfloat16
    x = np.asarray(x, np.float32).reshape(L, C)
    xT = np.ascontiguousarray(x.T).astype(bf16)
    Wq, Wk, Wv, Wo = (np.asarray(w, np.float32) for w in (Wq, Wk, Wv, Wo))
    in_maps = []
    for c in range(N_CORES):
        cols = slice(128 * c, 128 * (c + 1))
        in_maps.append({
            "xT": xT,
            "wq": _chunk_major(Wq[:, cols]).astype(bf16),
            "wk": _chunk_major(Wk[:, cols]).astype(bf16),
            "wv": _chunk_major(Wv[:, cols]).astype(bf16),
            "wo": np.ascontiguousarray(Wo[cols, :]).astype(bf16),
        })
    return in_maps


def combine_results(results):
    acc = np.zeros((C, L), np.float32)
    for r in results:
        acc += np.asarray(r["outT"], np.float32)
    return np.ascontiguousarray(acc.T)[None].astype(np.float32)


def kernel(x, Wq, Wk, Wv, Wo):
    from concourse.bass_utils import run_bass_kernel_spmd
    nc = _get_nc()
    in_maps = make_in_maps(x, Wq, Wk, Wv, Wo)
    res = run_bass_kernel_spmd(nc, in_maps, core_ids=list(range(N_CORES)))
    return combine_results(res.results)


# revision 10
# speedup vs baseline: 1.0448x; 1.0158x over previous
"""Causal self-attention (B=1, L=4096, C=1024, H=16, D=64) on 8 TRN2 NeuronCores.

Sharding: head tensor-parallelism — each core owns 2 of the 16 heads.
Host passes per-core bf16 inputs: x transposed [C, L], the Wq/Wk/Wv column
slices and Wo row slice for the core's heads. Each core computes its partial
of out.T = Wo_local.T @ attn_local.T; the host sums the 8 partials.

Schedule (v2): the kernel is PE-bound (~175us of matmul streaming at the
2.4GHz full p-state), so everything is organized to keep the tensor engine
continuously fed and everyone else off its critical path:
  - x is DMA'd per 512-column L-block on two hwdge queues (SP + ACT), so
    the first projection starts ~4.5us in instead of after the full 8MB.
  - QKV projection + V-transpose work for block b+2 is emitted as filler
    tasks interleaved into attention block b, hiding projections inside
    the attention middle.
  - S^T for both heads of one k-block goes into a single 2-bank PSUM tile
    [128, 2, 512]; ONE ACT exp instruction covers both heads (144 instead
    of 288 activates), keeping the scalar engine comfortably ahead.
  - No pre-exp additive mask: exp runs unmasked (|scores/8| < ~8, no
    overflow) and the upper triangle of P^T is zeroed post-exp on the
    otherwise-idle Pool engine via affine_select.
  - O matmuls trail S/exp by 2 iterations (software pipeline) and each
    q-block's epilogue (normalize + Wo + store) is emitted inside the
    next block's first iterations, so the PE always has ready work.
  - softmax denominators: row 64 of the O accumulation (vaug ones row),
    reciprocal via the fast DVE approximation, broadcast via a bf16
    rank-1 matmul.
"""
import math
import sys
from collections import deque
from contextlib import ExitStack

import numpy as np

sys.path.insert(0, "/opt/trn_rl_repo")

import ml_dtypes  # noqa: E402

import concourse.bass as bass  # noqa: E402,F401
import concourse.mybir as mybir  # noqa: E402
import concourse.tile as tile  # noqa: E402
from concourse import bacc  # noqa: E402

FP32 = mybir.dt.float32
BF16 = mybir.dt.bfloat16

L, C, H, D = 4096, 1024, 16, 64
N_CORES = 8


def _build_nc():
    DH2, QB, KB = 128, 512, 128
    NQ = L // QB          # 8 q-blocks
    NCC = C // 128        # 8 contraction chunks
    SUB = QB // KB        # 4 k-blocks per q-block width
    NKB = L // KB         # 32 k-blocks
    scale = 1.0 / math.sqrt(D)
    Exp = mybir.ActivationFunctionType.Exp

    nc = bacc.Bacc("TRN2", target_bir_lowering=False, debug=False,
                   num_devices=N_CORES)
    xT = nc.declare_dram_parameter("xT", [C, L], BF16, isOutput=False)
    # [128, NCC*DH2] chunk-major (host pre-reshaped) -> one contiguous DMA
    wq = nc.declare_dram_parameter("wq", [128, C], BF16, isOutput=False)
    wk = nc.declare_dram_parameter("wk", [128, C], BF16, isOutput=False)
    wv = nc.declare_dram_parameter("wv", [128, C], BF16, isOutput=False)
    wo = nc.declare_dram_parameter("wo", [DH2, C], BF16, isOutput=False)
    outT = nc.declare_dram_parameter("outT", [C, L], BF16, isOutput=True)

    with tile.TileContext(nc) as tc, ExitStack() as ctx:
        big = ctx.enter_context(tc.tile_pool(name="big", bufs=1))
        work = ctx.enter_context(tc.tile_pool(name="work", bufs=4))
        ptp = ctx.enter_context(tc.tile_pool(name="ptp", bufs=6))
        # PSUM: st 2 bufs x 2 banks + o0/o1 1 buf x 1 bank + misc 2 x 1 = 8
        ps_st = ctx.enter_context(tc.tile_pool(name="ps_st", bufs=2,
                                               space="PSUM"))
        ps_o = ctx.enter_context(tc.tile_pool(name="ps_o", bufs=1,
                                              space="PSUM"))
        ps_m = ctx.enter_context(tc.tile_pool(name="ps_m", bufs=2,
                                              space="PSUM"))

        ident = big.tile([128, 128], BF16, tag="ident")
        nc.gpsimd.memset(ident[:], 0.0)
        nc.gpsimd.affine_select(out=ident[:], in_=ident[:],
                                compare_op=mybir.AluOpType.not_equal,
                                fill=1.0, base=0,
                                pattern=[[-1, 128]], channel_multiplier=1)
        ones64 = big.tile([1, D], BF16, tag="ones64")
        nc.gpsimd.memset(ones64[:], 1.0)

        xt_sb = big.tile([128, NCC, L], BF16, tag="xt")
        wq_sb = big.tile([128, NCC, DH2], BF16, tag="wq")
        wk_sb = big.tile([128, NCC, DH2], BF16, tag="wk")
        wv_sb = big.tile([128, NCC, DH2], BF16, tag="wv")
        wo_sb = big.tile([128, C], BF16, tag="wo")
        # weights + odd x-blocks on the ACT hwdge queue; even x-blocks and
        # output stores on the SP queue — two DMA queues run concurrently.
        for w_sb, w_dram in ((wq_sb, wq), (wk_sb, wk), (wv_sb, wv)):
            nc.scalar.dma_start(
                w_sb[:], w_dram.rearrange("p (n d) -> p n d", n=NCC))
        nc.scalar.dma_start(wo_sb[:], wo[:])
        xTr = xT.rearrange("(n p) l -> p n l", p=128)
        for b in range(NQ):
            eng = nc.sync if b % 2 == 0 else nc.scalar
            eng.dma_start(xt_sb[:, :, b * QB:(b + 1) * QB],
                          xTr[:, :, b * QB:(b + 1) * QB])

        qt2 = big.tile([128, L], BF16, tag="qt2")
        kt2 = big.tile([128, L], BF16, tag="kt2")
        vt2 = big.tile([128, L], BF16, tag="vt2")
        vaug = big.tile([128, NKB, 130], BF16, tag="vaug")
        nc.gpsimd.memset(vaug[:, :, 64:65], 1.0)
        nc.gpsimd.memset(vaug[:, :, 129:130], 1.0)

        def emit_proj(dst, w_sb, b):
            pp = ps_m.tile([128, QB], FP32, tag="m", name="pp")
            for c in range(NCC):
                nc.tensor.matmul(pp[:], w_sb[:, c, :],
                                 xt_sb[:, c, b * QB:(b + 1) * QB],
                                 start=(c == 0), stop=(c == NCC - 1))
            nc.vector.tensor_copy(dst[:, b * QB:(b + 1) * QB], pp[:])

        def emit_vtrans(b):
            for s in range(SUB):
                i = b * SUB + s
                trp = ps_m.tile([128, 128], BF16, tag="m", name="trp")
                nc.tensor.transpose(trp[:], vt2[:, i * KB:(i + 1) * KB],
                                    ident[:])
                nc.vector.tensor_copy(vaug[:, i, 0:64], trp[:, 0:64])
                nc.vector.tensor_copy(vaug[:, i, 65:129], trp[:, 64:128])

        tasks = deque()

        def push_tasks(b):
            tasks.append(lambda: emit_proj(kt2, wk_sb, b))
            tasks.append(lambda: emit_proj(qt2, wq_sb, b))
            tasks.append(lambda: emit_proj(vt2, wv_sb, b))
            tasks.append(lambda: emit_vtrans(b))

        for b in (0, 1):
            push_tasks(b)
        while tasks:
            tasks.popleft()()

        o_ps_cur = [None, None]

        def emit_epilogue(j):
            att2 = work.tile([128, QB], BF16, tag="att2")
            den2 = work.tile([65, QB], FP32, tag="den2")
            osbs = []
            for h in range(2):
                osb = work.tile([64, QB], FP32, tag="osb", name=f"osb{h}")
                nc.vector.tensor_copy(osb[:], o_ps_cur[h][0:64, :])
                nc.vector.tensor_copy(den2[h * 64:h * 64 + 1, :],
                                      o_ps_cur[h][64:65, :])  # frees bank
                osbs.append(osb)
            rc2 = work.tile([65, QB], FP32, tag="rc2")
            nc.vector.reciprocal(rc2[:], den2[:])
            for h in range(2):
                rcb = work.tile([1, QB], BF16, tag="rcb", name=f"rcb{h}")
                nc.vector.tensor_copy(rcb[:], rc2[h * 64:h * 64 + 1, :])
                bc = ps_m.tile([64, QB], FP32, tag="m", name="bc")
                nc.tensor.matmul(bc[:], ones64[:], rcb[:],
                                 start=True, stop=True)
                nc.vector.tensor_mul(att2[h * D:(h + 1) * D, :],
                                     osbs[h][:], bc[:])
            for cc in range(NCC):
                op = ps_m.tile([128, QB], FP32, tag="m", name="op")
                nc.tensor.matmul(op[:], wo_sb[:, cc * 128:(cc + 1) * 128],
                                 att2[:], start=True, stop=True)
                ot = work.tile([128, QB], BF16, tag="ot")
                if cc % 2 == 0:
                    nc.scalar.copy(ot[:], op[:])
                else:
                    nc.vector.tensor_copy(ot[:], op[:])
                nc.sync.dma_start(
                    outT[cc * 128:(cc + 1) * 128, j * QB:(j + 1) * QB], ot[:])

        prev_j = None
        for j in range(NQ):
            if j + 2 < NQ:
                push_tasks(j + 2)
            nk = (j + 1) * SUB
            step = max(1, nk // 4)
            pend = []
            o_ps_new = None

            def emit_O(item, last_i):
                oi, oc0, opt = item
                for h in range(2):
                    nc.tensor.matmul(o_ps_new[h][:, oc0:QB],
                                     vaug[:, oi, h * 65:h * 65 + 65],
                                     opt[:, h, oc0:QB],
                                     start=(oi == 0), stop=(oi == last_i))

            for i in range(nk):
                c0 = max(0, i - j * SUB) * KB
                st = ps_st.tile([128, 2, QB], FP32, tag="st")
                for h in range(2):
                    r0, r1 = h * D, (h + 1) * D
                    nc.tensor.matmul(st[:, h, c0:QB],
                                     kt2[r0:r1, i * KB:(i + 1) * KB],
                                     qt2[r0:r1, j * QB + c0:(j + 1) * QB],
                                     start=True, stop=True)
                pt = ptp.tile([128, 2, QB], BF16, tag="pt")
                nc.scalar.activation(pt[:, :, c0:QB], st[:, :, c0:QB], Exp,
                                     scale=scale)
                if i >= j * SUB:
                    for h in range(2):
                        nc.gpsimd.affine_select(
                            out=pt[:, h, c0:c0 + KB],
                            in_=pt[:, h, c0:c0 + KB],
                            compare_op=mybir.AluOpType.is_ge, fill=0.0,
                            base=0, pattern=[[1, KB]], channel_multiplier=-1)
                pend.append((i, c0, pt))
                # previous block's epilogue before this block's first O
                # acquisition: ps_o has bufs=1, so the WAR order matters.
                if i == 1 and prev_j is not None:
                    emit_epilogue(prev_j)
                if len(pend) > 2:
                    if o_ps_new is None:
                        o_ps_new = [ps_o.tile([65, QB], FP32, tag=f"o{h}",
                                              name=f"o_ps{h}")
                                    for h in range(2)]
                    emit_O(pend.pop(0), nk - 1)
                if tasks and i % step == step - 1:
                    tasks.popleft()()
            for item in pend:
                emit_O(item, nk - 1)
            o_ps_cur = o_ps_new
            prev_j = j
        emit_epilogue(NQ - 1)
    nc.compile()
    return nc


_NC_CACHE = None


def _get_nc():
    global _NC_CACHE
    if _NC_CACHE is None:
        _NC_CACHE = _build_nc()
    return _NC_CACHE


def _chunk_major(w):
    """[1024, 128] -> [128, 8*128]: element [p, n*128+d] = w[n*128+p, d]."""
    return np.ascontiguousarray(
        w.reshape(8, 128, 128).transpose(1, 0, 2).reshape(128, 1024))


def make_in_maps(x, Wq, Wk, Wv, Wo):
    bf16 = ml_dtypes.bfloat16
    x = np.asarray(x, np.float32).reshape(L, C)
    xT = np.ascontiguousarray(x.T).astype(bf16)
    Wq, Wk, Wv, Wo = (np.asarray(w, np.float32) for w in (Wq, Wk, Wv, Wo))
    in_maps = []
    for c in range(N_CORES):
        cols = slice(128 * c, 128 * (c + 1))
        in_maps.append({
            "xT": xT,
            "wq": _chunk_major(Wq[:, cols]).astype(bf16),
            "wk": _chunk_major(Wk[:, cols]).astype(bf16),
            "wv": _chunk_major(Wv[:, cols]).astype(bf16),
            "wo": np.ascontiguousarray(Wo[cols, :]).astype(bf16),
        })
    return in_maps


def combine_results(results):
    acc = np.zeros((C, L), np.float32)
    for r in results:
        acc += np.asarray(r["outT"], np.float32)
    return np.ascontiguousarray(acc.T)[None].astype(np.float32)


def kernel(x, Wq, Wk, Wv, Wo):
    from concourse.bass_utils import run_bass_kernel_spmd
    nc = _get_nc()
    in_maps = make_in_maps(x, Wq, Wk, Wv, Wo)
    res = run_bass_kernel_spmd(nc, in_maps, core_ids=list(range(N_CORES)))
    return combine_results(res.results)
